# revision 1
# baseline (speedup 1.0000x reference)
"""Trainium2 Bass kernel: row-wise argmax over the vocab axis.

Problem: output = argmax(softmax(x, axis=2), axis=2)[..., None].astype(f32)
for x of shape (16, 512, 32000) f32. Softmax is monotone, so this is a plain
argmax over the last axis.

Sharding: data-parallel over the batch axis — core c handles batches
[2c, 2c+2), i.e. a (1024, 32000) f32 slab per core (131 MB, streamed once).

Per-core algorithm (memory-bound; one DVE pass over the data):
  1. Stream each 128-row tile as two 16000-wide chunks (64 KB contiguous per
     partition per DMA — big segments measured ~35 us faster end-to-end than
     3200-wide chunks on real HBM); tensor_reduce(max) over 128-wide blocks
     -> per-row block maxima [128, 250].
  2. vector.max + max_index over the 250 block maxima -> top-1 value and
     winning block id per row.
  3. Indirect-DMA gather of each row's winning 128-wide block from HBM.
  4. max_index over the gathered block -> in-block offset.
  5. final index = block_id * 128 + offset, cast to f32, DMA out.
Steps 4-5 are software-pipelined one tile behind steps 1-3 (and pinned there
with an ordering dep) so the in-order Vector engine never stalls on the
in-flight gather DMA mid-stream; measured ~420 us/core steady-state vs the
~366 us HBM-bandwidth floor (131 MB @ ~358 GB/s per core).
"""

import numpy as np

P = 128          # SBUF partitions / rows per tile
V = 32000        # vocab (reduced axis)
B = 128          # stage-1 block width (gather granularity)
CHUNK = 16000    # free-dim chunk per DMA/reduce (64 KB/partition segments)
BUFS = 2         # chunk buffering depth (2 x 64 KB per partition)
N_CORES = 8
ROWS_PER_CORE = 16 * 512 // N_CORES  # 1024

_cache = {}


TAPER_LAST = [12032, 3968]  # split of the last tile's final chunk (see below)


def _build(rows, repeat=1, chunk=CHUNK, bufs=BUFS, b=B, alternate_dma=False,
           defer_tail=True, taper_last=TAPER_LAST):
    import concourse.bass as bass
    import concourse.bacc as bacc
    import concourse.mybir as mybir
    from concourse.tile import TileContext

    f32 = mybir.dt.float32
    i32 = mybir.dt.int32
    u32 = mybir.dt.uint32

    nch = V // chunk
    cb = chunk // b     # blocks per chunk
    nb = V // b         # blocks per row
    assert chunk * nch == V and b * cb == chunk and b * nb == V

    nc = bacc.Bacc(trn_type="TRN2", debug=False)
    x = nc.dram_tensor("x", [rows, V], f32, kind="ExternalInput")
    y = nc.dram_tensor("y", [rows, 1], f32, kind="ExternalOutput")
    x_ap = x.ap()
    x_blocks = x_ap.rearrange("r (n b) -> (r n) b", b=b)  # [rows*nb, b]
    n_tiles = rows // P

    with TileContext(nc) as tc:
        with (
            tc.tile_pool(name="data", bufs=bufs) as dpool,
            tc.tile_pool(name="small", bufs=3) as spool,
            tc.tile_pool(name="cst", bufs=1) as cpool,
        ):
            # rowbase[p, 0] = p * nb  (block-row base within a tile's view)
            rowbase = cpool.tile([P, 1], i32)
            nc.gpsimd.iota(rowbase[:], [[1, 1]], base=0, channel_multiplier=nb)

            from concourse.tile import add_dep_helper

            def tail(t, top8, blk8, gath, after=None):
                """Consume the gathered winning block -> final f32 index."""
                inb8 = spool.tile([P, 8], u32, tag="inb8")
                mi = nc.vector.max_index(
                    out=inb8[:], in_max=top8[:], in_values=gath[:]
                )
                if after is not None:
                    # pin the gather-consuming op after the newest reduce so
                    # the scheduler cannot hoist it into the streaming stretch
                    add_dep_helper(mi.ins, after.ins, sync=False,
                                   reason="tail after current tile reduces")
                # final = block_id * b + in_block_offset, in f32 (exact: < 2^24)
                fblk = spool.tile([P, 1], f32, tag="fblk")
                finb = spool.tile([P, 1], f32, tag="finb")
                nc.vector.tensor_copy(out=fblk[:], in_=blk8[:, 0:1])
                nc.vector.tensor_copy(out=finb[:], in_=inb8[:, 0:1])
                res = spool.tile([P, 1], f32, tag="res")
                nc.vector.scalar_tensor_tensor(
                    out=res[:],
                    in0=fblk[:],
                    scalar=float(b),
                    in1=finb[:],
                    op0=mybir.AluOpType.mult,
                    op1=mybir.AluOpType.add,
                )
                # scalar-engine HWDGE ring: a waiting store never head-blocks
                # the SP ring that feeds the streaming chunk loads
                nc.scalar.dma_start(out=y.ap()[t * P:(t + 1) * P, :], in_=res[:])

            # Chunk widths per tile: uniform big chunks, except the LAST tile
            # may split its final chunk so the last reduce (which serializes
            # after the final byte of the stream) is short. Pieces stay large
            # (multiples of b, >= ~8 KB/partition) — tiny DMAs cost more in
            # issue latency than the reduce they save.
            uniform = [chunk] * nch
            if taper_last:
                pieces = [(p // b) * b for p in taper_last]
                assert sum(pieces) == chunk and all(p > 0 for p in pieces), pieces
                tapered = uniform[:-1] + pieces
            else:
                tapered = uniform
            assert sum(tapered) == V and all(c % b == 0 for c in tapered)

            pending = []
            for rep in range(repeat):
                for t in range(n_tiles):
                    blockmax = spool.tile([P, nb], f32, tag="blockmax")
                    last_reduce = None
                    col = 0
                    for w in (tapered if t == n_tiles - 1 else uniform):
                        ch = dpool.tile([P, chunk], f32, tag="chunk")
                        dma_eng = nc.scalar if (alternate_dma and col % (2 * chunk)) else nc.sync
                        dma_eng.dma_start(
                            out=ch[:, :w],
                            in_=x_ap[t * P:(t + 1) * P, col:col + w],
                        )
                        last_reduce = nc.vector.tensor_reduce(
                            out=blockmax[:, col // b:(col + w) // b],
                            in_=ch[:, :w].rearrange("p (n b) -> p n b", b=b),
                            axis=mybir.AxisListType.X,
                            op=mybir.AluOpType.max,
                        )
                        col += w

                    top8 = spool.tile([P, 8], f32, tag="top8")
                    blk8 = spool.tile([P, 8], u32, tag="blk8")
                    gath = spool.tile([P, b], f32, tag="gath")
                    gidx = spool.tile([P, 1], i32, tag="gidx")
                    nc.vector.max(out=top8[:], in_=blockmax[:])
                    nc.vector.max_index(
                        out=blk8[:], in_max=top8[:], in_values=blockmax[:]
                    )
                    # winning block, as an index into x_blocks local to this tile
                    nc.vector.tensor_tensor(
                        out=gidx[:],
                        in0=rowbase[:],
                        in1=blk8[:, 0:1].bitcast(i32),
                        op=mybir.AluOpType.add,
                    )
                    nc.gpsimd.indirect_dma_start(
                        out=gath[:],
                        out_offset=None,
                        in_=x_blocks,
                        in_offset=bass.IndirectOffsetOnAxis(ap=gidx[:, 0:1], axis=0),
                        element_offset=t * P * V,
                    )
                    if defer_tail:
                        # one-tile software pipeline: consume tile t-1's gather
                        # while tile t+1 streams, so DVE never stalls on the
                        # in-flight gather DMA
                        pending.append((t, top8, blk8, gath))
                        if len(pending) > 1:
                            tail(*pending.pop(0), after=last_reduce)
                    else:
                        tail(t, top8, blk8, gath)

                for args in pending:
                    tail(*args)
                pending = []
    nc.compile()
    return nc


def get_nc(rows=ROWS_PER_CORE, repeat=1):
    key = (rows, repeat)
    if key not in _cache:
        _cache[key] = _build(rows, repeat)
    return _cache[key]


def kernel(output: np.ndarray) -> np.ndarray:
    """Full-input entry point: (16, 512, 32000) f32 -> (16, 512, 1) f32."""
    from concourse.bass_utils import run_bass_kernel_spmd

    n, d, v = output.shape
    assert (n, d, v) == (16, 512, V), (n, d, v)
    x = np.ascontiguousarray(output, dtype=np.float32).reshape(
        N_CORES, ROWS_PER_CORE, V
    )
    nc = get_nc(ROWS_PER_CORE)
    in_maps = [{"x": x[c]} for c in range(N_CORES)]
    res = run_bass_kernel_spmd(nc, in_maps, core_ids=list(range(N_CORES)))
    out = np.stack([res.results[c]["y"] for c in range(N_CORES)], axis=0)
    return out.reshape(n, d, 1).astype(np.float32)



# revision 18
# speedup vs baseline: 1.2731x; 1.2731x over previous
"""Trainium2 Bass kernel: row-wise argmax over the vocab axis.

Problem: output = argmax(softmax(x, axis=2), axis=2)[..., None].astype(f32)
for x of shape (16, 512, 32000) f32. Softmax is monotone, so this is a plain
argmax over the last axis.

Sharding: data-parallel over the batch axis — core c handles batches
[2c, 2c+2), i.e. a (1024, 32000) f32 slab per core (131 MB, streamed once).

Per-core algorithm (memory-bound; one DVE pass over the data):
  1. Stream each 128-row tile as two 16000-wide chunks (64 KB contiguous per
     partition per DMA = single max-size DMA packets, triple-buffered);
     tensor_reduce(max) over 32-wide blocks -> per-row block maxima
     [128, 1000].
  2. vector.max + max_index over the block maxima -> top-1 value and
     winning block id per row.
  3. Indirect-DMA gather of each row's winning 32-wide block from HBM
     (128 B/row; small because the gather's engine dwell is pure overhead).
  4. max_index over the gathered block -> in-block offset.
  5. final index = block_id * 32 + offset, f32, into column t of a [128, 8]
     per-rep result tile.
  6. Once per rep: 32x32 stream-transpose of the result tile + 4 stores of
     8 x 128 B contiguous segments. (Naive per-tile [128,1] stores emit
     128 x 4 B HBM-write descriptors each and cost +38 us/rep — the single
     biggest overhead found on real HW.)
Steps 4-5 are software-pipelined one tile behind steps 1-3 — across rep
boundaries — and pinned with an ordering dep so the in-order Vector engine
never stalls on the in-flight gather DMA mid-stream.

Measured (NTFF-traced, single core, k=9 repeat steady-state): 303.9 us/rep
vs 302.4 us for a pure-DMA streaming kernel = 99.5% of the per-core DMA
roofline (16 DMA engines fanned out by one HWDGE queue; more queues don't
add bandwidth). The previous version measured 372.0 us on the same
instrument (harness-reported 320.5 us).
"""

import numpy as np

P = 128          # SBUF partitions / rows per tile
V = 32000        # vocab (reduced axis)
B = 32           # stage-1 block width (gather granularity)
CHUNK = 16000    # free-dim chunk per DMA/reduce (64 KB/partition segments)
BUFS = 3         # chunk buffering depth (3 x 64 KB per partition)
N_CORES = 8
ROWS_PER_CORE = 16 * 512 // N_CORES  # 1024

_cache = {}


TAPER_LAST = None  # optionally split the last tile's final chunk (see below)


def _build(rows, repeat=1, chunk=CHUNK, bufs=BUFS, b=B, alternate_dma=False,
           defer_tail=True, taper_last=TAPER_LAST):
    # NOTE: do NOT try to batch the per-tile gathers into one indirect DMA
    # with multiple offsets per partition: HW SWDGE honors only ONE offset
    # per partition per indirect DMA (it gathers out-row-width elements from
    # it), unlike the interp's ravel-all-indices model.
    import concourse.bass as bass
    import concourse.bacc as bacc
    import concourse.mybir as mybir
    from concourse.tile import TileContext

    f32 = mybir.dt.float32
    i32 = mybir.dt.int32
    u32 = mybir.dt.uint32

    nch = V // chunk
    cb = chunk // b     # blocks per chunk
    nb = V // b         # blocks per row
    assert chunk * nch == V and b * cb == chunk and b * nb == V

    nc = bacc.Bacc(trn_type="TRN2", debug=False)
    x = nc.dram_tensor("x", [rows, V], f32, kind="ExternalInput")
    y = nc.dram_tensor("y", [rows, 1], f32, kind="ExternalOutput")
    x_ap = x.ap()
    x_blocks = x_ap.rearrange("r (n b) -> (r n) b", b=b)  # [rows*nb, b]
    n_tiles = rows // P

    with TileContext(nc) as tc:
        with (
            tc.tile_pool(name="data", bufs=bufs) as dpool,
            tc.tile_pool(name="small", bufs=3) as spool,
            tc.tile_pool(name="cst", bufs=1) as cpool,
        ):
            # rowbase[p, 0] = p * nb  (block-row base within a tile's view)
            rowbase = cpool.tile([P, 1], i32)
            nc.gpsimd.iota(rowbase[:], [[1, 1]], base=0, channel_multiplier=nb)

            from concourse.tile import add_dep_helper

            def tail(t, top8, blk8, gath, resall, after=None):
                """Consume the gathered winning block -> final f32 index,
                written into column t%8 of the rep's [P, 8] result tile
                (stored once per rep — per-tile [128,1] stores cost ~38 us/rep
                in 4-byte HBM write descriptors)."""
                inb8 = spool.tile([P, 8], u32, tag="inb8")
                mi = nc.vector.max_index(
                    out=inb8[:], in_max=top8[:], in_values=gath[:]
                )
                if after is not None:
                    # pin the gather-consuming op after the newest reduce so
                    # the scheduler cannot hoist it into the streaming stretch
                    add_dep_helper(mi.ins, after.ins, sync=False,
                                   reason="tail after current tile reduces")
                # final = block_id * b + in_block_offset, in f32 (exact: < 2^24)
                fblk = spool.tile([P, 1], f32, tag="fblk")
                finb = spool.tile([P, 1], f32, tag="finb")
                nc.vector.tensor_copy(out=fblk[:], in_=blk8[:, 0:1])
                nc.vector.tensor_copy(out=finb[:], in_=inb8[:, 0:1])
                nc.vector.scalar_tensor_tensor(
                    out=resall[:, t % 8:t % 8 + 1],
                    in0=fblk[:],
                    scalar=float(b),
                    in1=finb[:],
                    op0=mybir.AluOpType.mult,
                    op1=mybir.AluOpType.add,
                )

            def store_rep(resall):
                """32x32 stream-transpose the rep's results, then ONE store of
                32 x 128B contiguous segments (vs 1024 x 4B without the
                transpose)."""
                trans = spool.tile([P, 32], f32, tag="trans")
                nc.vector.transpose(out=trans[:], in_=resall[:])
                dst = y.ap().rearrange("(t g j) o -> g t (j o)", g=4, j=32)
                # one DMA per 32-partition group: the DGE only honors a single
                # partition-level AP dim, so a fused (g u) src AP mis-lowers.
                # scalar-engine HWDGE ring: a waiting store never head-blocks
                # the SP ring that feeds the streaming chunk loads.
                for g in range(4):
                    nc.scalar.dma_start(out=dst[g], in_=trans[32 * g:32 * g + 8, :])

            # Chunk widths per tile: uniform big chunks, except the LAST tile
            # may split its final chunk so the last reduce (which serializes
            # after the final byte of the stream) is short. Pieces stay large
            # (multiples of b, >= ~8 KB/partition) — tiny DMAs cost more in
            # issue latency than the reduce they save.
            uniform = [chunk] * nch
            if taper_last:
                pieces = [(p // b) * b for p in taper_last]
                assert sum(pieces) == chunk and all(p > 0 for p in pieces), pieces
                tapered = uniform[:-1] + pieces
            else:
                tapered = uniform
            assert sum(tapered) == V and all(c % b == 0 for c in tapered)

            pending = []
            for rep in range(repeat):
                resall = spool.tile([P, 32], f32, tag="resall")
                nc.vector.memset(resall[:], 0.0)
                for t in range(n_tiles):
                    blockmax = spool.tile([P, nb], f32, tag="blockmax")
                    last_reduce = None
                    col = 0
                    for w in (tapered if t == n_tiles - 1 else uniform):
                        ch = dpool.tile([P, chunk], f32, tag="chunk")
                        dma_eng = nc.scalar if (alternate_dma and col % (2 * chunk)) else nc.sync
                        dma_eng.dma_start(
                            out=ch[:, :w],
                            in_=x_ap[t * P:(t + 1) * P, col:col + w],
                        )
                        last_reduce = nc.vector.tensor_reduce(
                            out=blockmax[:, col // b:(col + w) // b],
                            in_=ch[:, :w].rearrange("p (n b) -> p n b", b=b),
                            axis=mybir.AxisListType.X,
                            op=mybir.AluOpType.max,
                        )
                        col += w

                    top8 = spool.tile([P, 8], f32, tag="top8")
                    blk8 = spool.tile([P, 8], u32, tag="blk8")
                    gath = spool.tile([P, b], f32, tag="gath")
                    gidx = spool.tile([P, 1], i32, tag="gidx")
                    nc.vector.max(out=top8[:], in_=blockmax[:])
                    nc.vector.max_index(
                        out=blk8[:], in_max=top8[:], in_values=blockmax[:]
                    )
                    # winning block, as an index into x_blocks local to this tile
                    nc.vector.tensor_tensor(
                        out=gidx[:],
                        in0=rowbase[:],
                        in1=blk8[:, 0:1].bitcast(i32),
                        op=mybir.AluOpType.add,
                    )
                    nc.gpsimd.indirect_dma_start(
                        out=gath[:],
                        out_offset=None,
                        in_=x_blocks,
                        in_offset=bass.IndirectOffsetOnAxis(ap=gidx[:, 0:1], axis=0),
                        element_offset=t * P * V,
                    )
                    if defer_tail:
                        # one-tile software pipeline ACROSS rep boundaries:
                        # consume tile t-1's gather while tile t+1 streams, so
                        # the in-order Vector engine never stalls on the
                        # in-flight gather DMA — including the last tile of a
                        # rep, whose tail (and the rep's store) runs under the
                        # next rep's tile-0 streaming window.
                        pending.append((t, top8, blk8, gath, resall))
                        if len(pending) > 1:
                            args = pending.pop(0)
                            tail(*args, after=last_reduce)
                            if args[0] == n_tiles - 1:
                                store_rep(args[4])
                    else:
                        tail(t, top8, blk8, gath, resall)
                        if t == n_tiles - 1:
                            store_rep(resall)

            # drain the cross-rep pipeline after the final rep
            for args in pending:
                tail(*args)
                if args[0] == n_tiles - 1:
                    store_rep(args[4])
            pending = []
    nc.compile()
    return nc


def get_nc(rows=ROWS_PER_CORE, repeat=1):
    key = (rows, repeat)
    if key not in _cache:
        _cache[key] = _build(rows, repeat)
    return _cache[key]


def kernel(output: np.ndarray) -> np.ndarray:
    """Full-input entry point: (16, 512, 32000) f32 -> (16, 512, 1) f32."""
    from concourse.bass_utils import run_bass_kernel_spmd

    n, d, v = output.shape
    assert (n, d, v) == (16, 512, V), (n, d, v)
    x = np.ascontiguousarray(output, dtype=np.float32).reshape(
        N_CORES, ROWS_PER_CORE, V
    )
    nc = get_nc(ROWS_PER_CORE)
    in_maps = [{"x": x[c]} for c in range(N_CORES)]
    res = run_bass_kernel_spmd(nc, in_maps, core_ids=list(range(N_CORES)))
    out = np.stack([res.results[c]["y"] for c in range(N_CORES)], axis=0)
    return out.reshape(n, d, 1).astype(np.float32)

